# revision 8
# baseline (speedup 1.0000x reference)
"""Trainium2 Bass kernel for nn_Model_17085379903564 (HiPPO-LegT multiscale
spectral forecaster).

Math: with instance-normed input f[t] (per (b,e) series), the reference does
  cs[t]   = cs[t-1] @ Ad^T + f[t]*Bd          (scan, collects all states)
  xf      = rfft_t(cs)[..., :32]
  outf    = einsum('bik,iok->bok', xf, w)     (w = spec_w_real + i spec_w_imag)
  xdc     = irfft(pad(outf))[..., 511]
  dec_s   = xdc @ Em_s[-512:].T
  out     = (w0*dec_0 + w1*dec_1 + b) * std + mean

Because cs is a causal convolution of f with G[m] = Ad^m Bd, and only 32 DFT
modes (and one irfft sample) are consumed, everything between f and xdc is a
LINEAR map with constant coefficients:

  e_k := (2 - [k=0])/L * exp(+2i pi k*511/L)       (point-irfft weights)
  Exf[be,n,k] := e_k * xf[be,n,k] = sum_t f[t,be] * W2[t,n,k]
  W2[t,n,k] = e_k * z_k^t * sum_{m<=L-1-t} z_k^m G[m,n],  z_k = exp(-2i pi k/L)
  xdc[be,o] = sum_{n,k} Re(Exf)*Re(w) - Im(Exf)*Im(w)

So per scale: one (BE=128, L) x (L, N*32) matmul pair (re/im) plus one
(BE, N*32) x (N*32, N) contraction pair. No scan, no FFT.

Sharding (8 cores): stage 1+2 sharded over n (32 rows of N=256 each, i.e.
1/8 of W2 and of w per core) -> each core holds a partial xdc (128,512);
one 256 KB AllReduce; stage 3 (Em matmul + un-norm) sharded over the output
horizon P (64 of 512 per core). x and the tiny mlp tensors are replicated.
"""

import numpy as np

import concourse.bass as bass
import concourse.bacc as bacc
import concourse.mybir as mybir
import concourse.tile as tile
from concourse.bass_utils import run_bass_kernel_spmd
from concourse.masks import make_identity

# ---- problem constants (hardcoded; kernel.py must be self-contained) ----
B_SZ = 4
SEQ_LEN = 1024
PRED_LEN = 512
E_IN = 32
N_ORD = 256
MODES = 32
MULTISCALE = (1, 2)
BE = B_SZ * E_IN            # 128
N_CORES = 8
NSL = N_ORD // N_CORES      # 32  n-rows per core
NK = NSL * MODES            # 1024 contraction length of stage 2 per core
PSL = PRED_LEN // N_CORES   # 64  output horizon slice per core

F32 = mybir.dt.float32
# dtype of the two big matmul stages (f32 = exact-ish, bf16 = half the DMA)
MM_DT = F32
MM_NP = np.float32 if MM_DT == F32 else np.dtype("bfloat16")


# ---------------------------------------------------------------- constants
def _transition_lmu(N):
    Q = np.arange(N, dtype=np.float64)
    R = (2 * Q + 1)[:, None]
    j, i = np.meshgrid(Q, Q)
    A = np.where(i < j, -1.0, (-1.0) ** (i - j + 1)) * R
    Bv = ((-1.0) ** Q[:, None] * R)[:, 0]
    return A, Bv


def _bilinear(A, Bv, dt):
    I = np.eye(A.shape[0])
    M = I - (dt / 2.0) * A
    Ad = np.linalg.solve(M, I + (dt / 2.0) * A)
    Bd = np.linalg.solve(M, dt * Bv)
    return Ad, Bd


def _legendre_vander(x, N):
    P = np.zeros((N, x.shape[0]))
    P[0] = 1.0
    if N > 1:
        P[1] = x
    for n in range(1, N - 1):
        P[n + 1] = ((2 * n + 1) * x * P[n] - n * P[n - 1]) / (n + 1)
    return P.T


def _scale_consts(ms):
    """Returns (W2re, W2imneg, EmT) for one scale.

    W2re/W2imneg: (L, N_ORD, MODES) f64 — e_k-folded stage-1 operator
    EmT: (N_ORD, PRED_LEN) f64 — Em[-512:].T
    """
    L = ms * PRED_LEN
    A, Bv = _transition_lmu(N_ORD)
    Ad, Bd = _bilinear(A, Bv, 1.0 / L)
    vals = np.arange(0.0, 1.0, 1.0 / L)
    Em = _legendre_vander(1.0 - 2.0 * vals, N_ORD)        # (L, N)

    G = np.empty((L, N_ORD))
    g = Bd.copy()
    for m in range(L):
        G[m] = g
        g = Ad @ g
    k = np.arange(MODES)
    z = np.exp(-2j * np.pi * k / L)                       # (32,)
    zm = z[None, :] ** np.arange(L)[:, None]              # (L, 32)
    Gpre = np.cumsum(zm[:, None, :] * G[:, :, None], axis=0)   # (L, N, 32)
    W = zm[:, None, :] * Gpre[::-1]                       # (L, N, 32) complex
    e = (2.0 - (k == 0)) / L * np.exp(2j * np.pi * k * (PRED_LEN - 1) / L)
    W2 = W * e[None, None, :]
    return W2.real, -W2.imag, Em[-PRED_LEN:].T


_CONSTS = None


def _get_consts():
    global _CONSTS
    if _CONSTS is None:
        _CONSTS = [_scale_consts(ms) for ms in MULTISCALE]
    return _CONSTS


# ---------------------------------------------------------------- bass prog
def _build_nc():
    L0, L1 = PRED_LEN, 2 * PRED_LEN          # 512, 1024
    nc = bacc.Bacc("TRN2", target_bir_lowering=False, debug=False,
                   num_devices=N_CORES)

    xt = nc.declare_dram_parameter("xt", [BE, SEQ_LEN], F32, isOutput=False)
    w2 = {}
    for s, L in ((0, L0), (1, L1)):
        for part in ("re", "im"):
            w2[s, part] = nc.declare_dram_parameter(
                f"w2{part}{s}", [L, NK], MM_DT, isOutput=False)
    wsp = {}
    for s in (0, 1):
        for part in ("re", "im"):
            wsp[s, part] = nc.declare_dram_parameter(
                f"w{part}{s}", [NK, N_ORD], MM_DT, isOutput=False)
    emt = [nc.declare_dram_parameter(f"emt{s}", [N_ORD, PSL], F32,
                                     isOutput=False) for s in (0, 1)]
    mlpw = nc.declare_dram_parameter("mlpw", [1, 2], F32, isOutput=False)
    mlpb = nc.declare_dram_parameter("mlpb", [1, 1], F32, isOutput=False)
    out_dec = nc.declare_dram_parameter("out_dec", [BE, PSL], F32,
                                        isOutput=True)

    with tile.TileContext(nc, num_cores=N_CORES) as tc:
        _emit(nc, tc, xt, w2, wsp, emt, mlpw, mlpb, out_dec)
    nc.finalize()
    return nc


def _emit(nc, tc, xt, w2, wsp, emt, mlpw, mlpb, out_dec):
    L1CH = SEQ_LEN // 128                    # 8 time chunks of the full xn
    from contextlib import ExitStack
    with ExitStack() as ctx:
        const = ctx.enter_context(tc.tile_pool(name="const", bufs=1))
        work = ctx.enter_context(tc.tile_pool(name="work", bufs=1))
        w2pool = ctx.enter_context(tc.tile_pool(name="w2", bufs=2))
        wpool = ctx.enter_context(tc.tile_pool(name="wsp", bufs=1))
        ps_tr = ctx.enter_context(
            tc.tile_pool(name="ps_tr", bufs=2, space="PSUM"))
        ps_exf = ctx.enter_context(
            tc.tile_pool(name="ps_exf", bufs=2, space="PSUM"))
        ps_small = ctx.enter_context(
            tc.tile_pool(name="ps_small", bufs=1, space="PSUM"))
        dram = ctx.enter_context(tc.tile_pool(name="dram", bufs=1,
                                              space="DRAM"))

        ident = const.tile([128, 128], F32, tag="ident")
        make_identity(nc, ident[:])

        # ---------------- phase A: instance norm (layout (be, t)) ---------
        xt_t = work.tile([BE, SEQ_LEN], F32, tag="xt")
        nc.sync.dma_start(xt_t[:], xt[:, :])

        sumx = work.tile([BE, 1], F32, tag="sumx")
        nc.vector.reduce_sum(sumx[:], xt_t[:], axis=mybir.AxisListType.X)
        sq = work.tile([BE, SEQ_LEN], F32, tag="sq")
        sumsq = work.tile([BE, 1], F32, tag="sumsq")
        nc.scalar.activation(sq[:], xt_t[:],
                             mybir.ActivationFunctionType.Square,
                             accum_out=sumsq[:])
        mean = work.tile([BE, 1], F32, tag="mean")
        nc.scalar.mul(mean[:], sumx[:], 1.0 / SEQ_LEN)
        ex2 = work.tile([BE, 1], F32, tag="ex2")
        nc.scalar.mul(ex2[:], sumsq[:], 1.0 / SEQ_LEN)
        m2 = work.tile([BE, 1], F32, tag="m2")
        nc.scalar.square(m2[:], mean[:])
        var = work.tile([BE, 1], F32, tag="var")
        nc.vector.tensor_sub(var[:], ex2[:], m2[:])
        eps = work.tile([BE, 1], F32, tag="eps")
        nc.vector.memset(eps[:], 1e-5)
        std = work.tile([BE, 1], F32, tag="std")
        nc.scalar.activation(std[:], var[:],
                             mybir.ActivationFunctionType.Sqrt, bias=eps[:])
        inv = work.tile([BE, 1], F32, tag="inv")
        nc.vector.reciprocal(inv[:], std[:])
        nbias = work.tile([BE, 1], F32, tag="nbias")
        nc.vector.tensor_mul(nbias[:], mean[:], inv[:])
        nc.scalar.mul(nbias[:], nbias[:], -1.0)

        xn = work.tile([BE, SEQ_LEN], F32, tag="xn")
        nc.scalar.activation(xn[:], xt_t[:],
                             mybir.ActivationFunctionType.Identity,
                             bias=nbias[:], scale=inv[:])

        # ---------------- phase B: transpose xn -> ft[j] = (t, be) --------
        ft = [work.tile([128, BE], MM_DT, tag=f"ft{j}", name=f"ft{j}")
              for j in range(L1CH)]
        for j in range(L1CH):
            pst = ps_tr.tile([128, 128], F32, tag="tr")
            nc.tensor.transpose(pst[:], xn[:, j * 128:(j + 1) * 128],
                                ident[:])
            nc.vector.tensor_copy(ft[j][:], pst[:])

        # ---------------- phase C: stages 1+2 per scale -------------------
        xdc = work.tile([BE, 2 * N_ORD], F32, tag="xdc")
        for s in (0, 1):
            L = (s + 1) * PRED_LEN
            lch = L // 128
            j0 = L1CH - lch                  # ft chunk offset for this scale
            # stage 1: Exf[be, nk] += ft[d].T @ W2[d]
            w2t = {}
            for part in ("re", "im"):
                w2t[part] = w2pool.tile([128, lch, NK], MM_DT,
                                        tag=f"w2{part}", name=f"w2t{part}")
                nc.sync.dma_start(
                    w2t[part][:],
                    w2[s, part].rearrange("(c p) f -> p c f", p=128))
            exf_sb = {}
            for part in ("re", "im"):
                exf_sb[part] = work.tile([BE, NK], F32, tag=f"exfsb{part}",
                                           name=f"exfsb{part}")
            for part in ("re", "im"):
                for h in (0, 1):
                    eps_t = ps_exf.tile([BE, 512], F32, tag="exf",
                                        name="exfps")
                    for d in range(lch):
                        nc.tensor.matmul(
                            eps_t[:],
                            lhsT=ft[j0 + d][:],
                            rhs=w2t[part][:, d, h * 512:(h + 1) * 512],
                            start=(d == 0), stop=(d == lch - 1))
                    nc.vector.tensor_copy(
                        exf_sb[part][:, h * 512:(h + 1) * 512], eps_t[:])
            # transpose Exf -> (nk, be) chunks
            exf_T = {}
            for part in ("re", "im"):
                for i in range(NK // 128):
                    pst = ps_tr.tile([128, 128], F32, tag="tr")
                    nc.tensor.transpose(
                        pst[:], exf_sb[part][:, i * 128:(i + 1) * 128],
                        ident[:])
                    t = work.tile([128, BE], MM_DT, tag=f"exfT{part}{i}",
                                  name=f"exfT{part}{i}")
                    nc.vector.tensor_copy(t[:], pst[:])
                    exf_T[part, i] = t
            # stage 2: xdc[be, o] += sum_i exf_T[i].T @ w[i]
            wt = {}
            for part in ("re", "im"):
                wt[part] = wpool.tile([128, NK // 128, N_ORD], MM_DT,
                                      tag=f"w{part}", name=f"wt{part}")
                nc.sync.dma_start(
                    wt[part][:],
                    wsp[s, part].rearrange("(c p) f -> p c f", p=128))
            xdc_ps = ps_small.tile([BE, N_ORD], F32, tag="xdc")
            nmm = NK // 128
            for i in range(nmm):
                for part in ("re", "im"):
                    nc.tensor.matmul(
                        xdc_ps[:],
                        lhsT=exf_T[part, i][:],
                        rhs=wt[part][:, i, :],
                        start=(i == 0 and part == "re"),
                        stop=(i == nmm - 1 and part == "im"))
            nc.vector.tensor_copy(xdc[:, s * N_ORD:(s + 1) * N_ORD],
                                  xdc_ps[:])

        # ---------------- phase D: AllReduce partial xdc ------------------
        bounce_in = dram.tile([BE, 2 * N_ORD], F32, tag="bin")
        bounce_out = dram.tile([BE, 2 * N_ORD], F32, tag="bout")
        nc.gpsimd.dma_start(bounce_in[:], xdc[:])
        nc.gpsimd.collective_compute(
            "AllReduce",
            mybir.AluOpType.add,
            replica_groups=[list(range(N_CORES))],
            ins=[bounce_in.opt()],
            outs=[bounce_out.opt()],
        )
        xdcr = work.tile([BE, 2 * N_ORD], F32, tag="xdcr")
        nc.gpsimd.dma_start(xdcr[:], bounce_out[:])

        # ---------------- phase E: dec + un-norm --------------------------
        # broadcast mlp scalars across partitions via 1-row matmul
        mlpw_sb = const.tile([1, 2], F32, tag="mlpw")
        nc.sync.dma_start(mlpw_sb[:], mlpw[:, :])
        mlpb_sb = const.tile([1, 1], F32, tag="mlpb")
        nc.sync.dma_start(mlpb_sb[:], mlpb[:, :])
        ones = const.tile([1, 128], F32, tag="ones")
        nc.vector.memset(ones[:], 1.0)
        ps_w = ps_small.tile([128, 2], F32, tag="psmall")
        nc.tensor.matmul(ps_w[:], lhsT=ones[:], rhs=mlpw_sb[:])
        ws_sb = work.tile([128, 2], F32, tag="ws")
        nc.vector.tensor_copy(ws_sb[:], ps_w[:])
        ps_b = ps_small.tile([128, 1], F32, tag="psmall")
        nc.tensor.matmul(ps_b[:], lhsT=ones[:], rhs=mlpb_sb[:])
        bs_sb = work.tile([128, 1], F32, tag="bs")
        nc.vector.tensor_copy(bs_sb[:], ps_b[:])

        emt_sb = []
        for s in (0, 1):
            t = const.tile([128, 2, PSL], F32, tag=f"emt{s}", name=f"emtsb{s}")
            nc.sync.dma_start(t[:], emt[s].rearrange("(c p) f -> p c f",
                                                     p=128))
            emt_sb.append(t)

        # transpose xdc chunks, scaling by mlp_weight[s] on the way out
        dec_ps = ps_small.tile([BE, PSL], F32, tag="dec")
        first = True
        for s in (0, 1):
            for j in (0, 1):
                pst = ps_tr.tile([128, 128], F32, tag="tr")
                nc.tensor.transpose(
                    pst[:], xdcr[:, s * N_ORD + j * 128:
                                 s * N_ORD + (j + 1) * 128], ident[:])
                xt_sc = work.tile([128, BE], F32, tag="xdcT")
                nc.scalar.activation(xt_sc[:], pst[:],
                                     mybir.ActivationFunctionType.Copy,
                                     scale=ws_sb[:, s:s + 1])
                nc.tensor.matmul(dec_ps[:], lhsT=xt_sc[:],
                                 rhs=emt_sb[s][:, j, :],
                                 start=first, stop=(s == 1 and j == 1))
                first = False

        # out = dec * std + (mlp_bias * std + mean)
        bmu = work.tile([BE, 1], F32, tag="bmu")
        nc.vector.tensor_mul(bmu[:], bs_sb[:], std[:])
        nc.vector.tensor_add(bmu[:], bmu[:], mean[:])
        out_sb = work.tile([BE, PSL], F32, tag="out")
        nc.scalar.activation(out_sb[:], dec_ps[:],
                             mybir.ActivationFunctionType.Identity,
                             bias=bmu[:], scale=std[:])
        nc.sync.dma_start(out_dec[:, :], out_sb[:])


_NC = None


def _get_nc():
    global _NC
    if _NC is None:
        _NC = _build_nc()
    return _NC


# ---------------------------------------------------------------- host side
def _in_maps(x_enc, spec_w_real, spec_w_imag, mlp_weight, mlp_bias):
    consts = _get_consts()
    xt = np.ascontiguousarray(
        np.transpose(x_enc, (0, 2, 1)).reshape(BE, SEQ_LEN)).astype(
            np.float32, copy=False)
    mw = np.asarray(mlp_weight, np.float32).reshape(1, 2)
    mb = np.asarray(mlp_bias, np.float32).reshape(1, 1)
    maps = []
    for c in range(N_CORES):
        n0 = c * NSL
        m = {"xt": xt, "mlpw": mw, "mlpb": mb}
        for s in (0, 1):
            w2re, w2imneg, emT = consts[s]
            L = (s + 1) * PRED_LEN
            m[f"w2re{s}"] = np.ascontiguousarray(
                w2re[:, n0:n0 + NSL, :].reshape(L, NK)).astype(MM_NP)
            m[f"w2im{s}"] = np.ascontiguousarray(
                w2imneg[:, n0:n0 + NSL, :].reshape(L, NK)).astype(MM_NP)
            m[f"wre{s}"] = np.ascontiguousarray(
                spec_w_real[s, n0:n0 + NSL].transpose(0, 2, 1).reshape(
                    NK, N_ORD)).astype(MM_NP)
            m[f"wim{s}"] = np.ascontiguousarray(
                spec_w_imag[s, n0:n0 + NSL].transpose(0, 2, 1).reshape(
                    NK, N_ORD)).astype(MM_NP)
            m[f"emt{s}"] = np.ascontiguousarray(
                emT[:, c * PSL:(c + 1) * PSL]).astype(np.float32)
        maps.append(m)
    return maps


def kernel(x_enc, spec_w_real, spec_w_imag, mlp_weight, mlp_bias,
           _trace=False, _trace_kwargs=None):
    x_enc = np.asarray(x_enc, np.float32)
    spec_w_real = np.asarray(spec_w_real, np.float32)
    spec_w_imag = np.asarray(spec_w_imag, np.float32)
    maps = _in_maps(x_enc, spec_w_real, spec_w_imag, mlp_weight, mlp_bias)
    nc = _get_nc()
    res = run_bass_kernel_spmd(nc, maps, list(range(N_CORES)),
                               trace=_trace, **(_trace_kwargs or {}))
    parts = [res.results[c]["out_dec"].reshape(B_SZ, E_IN, PSL)
             for c in range(N_CORES)]
    out = np.concatenate(parts, axis=2).transpose(0, 2, 1)
    if _trace:
        return np.ascontiguousarray(out), res
    return np.ascontiguousarray(out)


# revision 9
# speedup vs baseline: 1.3858x; 1.3858x over previous
"""Trainium2 Bass kernel for nn_Model_17085379903564 (HiPPO-LegT multiscale
spectral forecaster).

Math: with instance-normed input f[t] (per (b,e) series), the reference does
  cs[t]   = cs[t-1] @ Ad^T + f[t]*Bd          (scan, collects all states)
  xf      = rfft_t(cs)[..., :32]
  outf    = einsum('bik,iok->bok', xf, w)     (w = spec_w_real + i spec_w_imag)
  xdc     = irfft(pad(outf))[..., 511]
  dec_s   = xdc @ Em_s[-512:].T
  out     = (w0*dec_0 + w1*dec_1 + b) * std + mean

Because cs is a causal convolution of f with G[m] = Ad^m Bd, and only 32 DFT
modes (and one irfft sample) are consumed, everything between f and xdc is a
LINEAR map with constant coefficients:

  e_k := (2 - [k=0])/L * exp(+2i pi k*511/L)       (point-irfft weights)
  Exf[be,n,k] := e_k * xf[be,n,k] = sum_t f[t,be] * W2[t,n,k]
  W2[t,n,k] = e_k * z_k^t * sum_{m<=L-1-t} z_k^m G[m,n],  z_k = exp(-2i pi k/L)
  xdc[be,o] = sum_{n,k} Re(Exf)*Re(w) - Im(Exf)*Im(w)

So per scale: one (BE=128, L) x (L, N*32) matmul pair (re/im) plus one
(BE, N*32) x (N*32, N) contraction pair. No scan, no FFT.

Sharding (8 cores): stage 1+2 sharded over n (32 rows of N=256 each, i.e.
1/8 of W2 and of w per core) -> each core holds a partial xdc (128,512);
one 256 KB AllReduce; stage 3 (Em matmul + un-norm) sharded over the output
horizon P (64 of 512 per core). x and the tiny mlp tensors are replicated.
"""

import ml_dtypes
import numpy as np

import concourse.bass as bass
import concourse.bacc as bacc
import concourse.mybir as mybir
import concourse.tile as tile
from concourse.bass_utils import run_bass_kernel_spmd
from concourse.masks import make_identity

# ---- problem constants (hardcoded; kernel.py must be self-contained) ----
B_SZ = 4
SEQ_LEN = 1024
PRED_LEN = 512
E_IN = 32
N_ORD = 256
MODES = 32
MULTISCALE = (1, 2)
BE = B_SZ * E_IN            # 128
N_CORES = 8
NSL = N_ORD // N_CORES      # 32  n-rows per core
NK = NSL * MODES            # 1024 contraction length of stage 2 per core
PSL = PRED_LEN // N_CORES   # 64  output horizon slice per core

F32 = mybir.dt.float32
BF16 = mybir.dt.bfloat16
# dtype of the two big matmul stages (f32 = exact-ish, bf16 = half the DMA
# and ~4x the PE rate: fp32 matmul lowers to LOW_HIGH double passes)
MM_DT = BF16
MM_NP = np.float32 if MM_DT == F32 else np.dtype(ml_dtypes.bfloat16)


# ---------------------------------------------------------------- constants
def _transition_lmu(N):
    Q = np.arange(N, dtype=np.float64)
    R = (2 * Q + 1)[:, None]
    j, i = np.meshgrid(Q, Q)
    A = np.where(i < j, -1.0, (-1.0) ** (i - j + 1)) * R
    Bv = ((-1.0) ** Q[:, None] * R)[:, 0]
    return A, Bv


def _bilinear(A, Bv, dt):
    I = np.eye(A.shape[0])
    M = I - (dt / 2.0) * A
    Ad = np.linalg.solve(M, I + (dt / 2.0) * A)
    Bd = np.linalg.solve(M, dt * Bv)
    return Ad, Bd


def _legendre_vander(x, N):
    P = np.zeros((N, x.shape[0]))
    P[0] = 1.0
    if N > 1:
        P[1] = x
    for n in range(1, N - 1):
        P[n + 1] = ((2 * n + 1) * x * P[n] - n * P[n - 1]) / (n + 1)
    return P.T


def _scale_consts(ms):
    """Returns (W2re, W2imneg, EmT) for one scale.

    W2re/W2imneg: (L, N_ORD, MODES) f64 — e_k-folded stage-1 operator
    EmT: (N_ORD, PRED_LEN) f64 — Em[-512:].T
    """
    L = ms * PRED_LEN
    A, Bv = _transition_lmu(N_ORD)
    Ad, Bd = _bilinear(A, Bv, 1.0 / L)
    vals = np.arange(0.0, 1.0, 1.0 / L)
    Em = _legendre_vander(1.0 - 2.0 * vals, N_ORD)        # (L, N)

    G = np.empty((L, N_ORD))
    g = Bd.copy()
    for m in range(L):
        G[m] = g
        g = Ad @ g
    k = np.arange(MODES)
    z = np.exp(-2j * np.pi * k / L)                       # (32,)
    zm = z[None, :] ** np.arange(L)[:, None]              # (L, 32)
    Gpre = np.cumsum(zm[:, None, :] * G[:, :, None], axis=0)   # (L, N, 32)
    W = zm[:, None, :] * Gpre[::-1]                       # (L, N, 32) complex
    e = (2.0 - (k == 0)) / L * np.exp(2j * np.pi * k * (PRED_LEN - 1) / L)
    W2 = W * e[None, None, :]
    return W2.real, -W2.imag, Em[-PRED_LEN:].T


_CONSTS = None


def _get_consts():
    global _CONSTS
    if _CONSTS is None:
        _CONSTS = [_scale_consts(ms) for ms in MULTISCALE]
    return _CONSTS


# ---------------------------------------------------------------- bass prog
def _build_nc():
    L0, L1 = PRED_LEN, 2 * PRED_LEN          # 512, 1024
    nc = bacc.Bacc("TRN2", target_bir_lowering=False, debug=False,
                   num_devices=N_CORES)

    xt = nc.declare_dram_parameter("xt", [BE, SEQ_LEN], F32, isOutput=False)
    w2 = {}
    for s, L in ((0, L0), (1, L1)):
        for part in ("re", "im"):
            w2[s, part] = nc.declare_dram_parameter(
                f"w2{part}{s}", [L, NK], MM_DT, isOutput=False)
    wsp = {}
    for s in (0, 1):
        for part in ("re", "im"):
            wsp[s, part] = nc.declare_dram_parameter(
                f"w{part}{s}", [NK, N_ORD], MM_DT, isOutput=False)
    emt = [nc.declare_dram_parameter(f"emt{s}", [N_ORD, PSL], F32,
                                     isOutput=False) for s in (0, 1)]
    mlpw = nc.declare_dram_parameter("mlpw", [1, 2], F32, isOutput=False)
    mlpb = nc.declare_dram_parameter("mlpb", [1, 1], F32, isOutput=False)
    out_dec = nc.declare_dram_parameter("out_dec", [BE, PSL], F32,
                                        isOutput=True)

    with tile.TileContext(nc, num_cores=N_CORES) as tc:
        _emit(nc, tc, xt, w2, wsp, emt, mlpw, mlpb, out_dec)
    nc.finalize()
    return nc


def _emit(nc, tc, xt, w2, wsp, emt, mlpw, mlpb, out_dec):
    L1CH = SEQ_LEN // 128                    # 8 time chunks of the full xn
    from contextlib import ExitStack
    with ExitStack() as ctx:
        const = ctx.enter_context(tc.tile_pool(name="const", bufs=1))
        work = ctx.enter_context(tc.tile_pool(name="work", bufs=1))
        w2pool = ctx.enter_context(tc.tile_pool(name="w2", bufs=2))
        wpool = ctx.enter_context(tc.tile_pool(name="wsp", bufs=1))
        ps_tr = ctx.enter_context(
            tc.tile_pool(name="ps_tr", bufs=2, space="PSUM"))
        ps_exf = ctx.enter_context(
            tc.tile_pool(name="ps_exf", bufs=2, space="PSUM"))
        ps_small = ctx.enter_context(
            tc.tile_pool(name="ps_small", bufs=1, space="PSUM"))
        dram = ctx.enter_context(tc.tile_pool(name="dram", bufs=1,
                                              space="DRAM"))

        ident = const.tile([128, 128], F32, tag="ident")
        make_identity(nc, ident[:])
        ident_mm = const.tile([128, 128], MM_DT, tag="ident_mm")
        make_identity(nc, ident_mm[:])

        # ---------------- phase A: instance norm (layout (be, t)) ---------
        xt_t = work.tile([BE, SEQ_LEN], F32, tag="xt")
        nc.sync.dma_start(xt_t[:], xt[:, :])

        sumx = work.tile([BE, 1], F32, tag="sumx")
        nc.vector.reduce_sum(sumx[:], xt_t[:], axis=mybir.AxisListType.X)
        sq = work.tile([BE, SEQ_LEN], F32, tag="sq")
        sumsq = work.tile([BE, 1], F32, tag="sumsq")
        nc.scalar.activation(sq[:], xt_t[:],
                             mybir.ActivationFunctionType.Square,
                             accum_out=sumsq[:])
        mean = work.tile([BE, 1], F32, tag="mean")
        nc.scalar.mul(mean[:], sumx[:], 1.0 / SEQ_LEN)
        ex2 = work.tile([BE, 1], F32, tag="ex2")
        nc.scalar.mul(ex2[:], sumsq[:], 1.0 / SEQ_LEN)
        m2 = work.tile([BE, 1], F32, tag="m2")
        nc.scalar.square(m2[:], mean[:])
        var = work.tile([BE, 1], F32, tag="var")
        nc.vector.tensor_sub(var[:], ex2[:], m2[:])
        eps = work.tile([BE, 1], F32, tag="eps")
        nc.vector.memset(eps[:], 1e-5)
        std = work.tile([BE, 1], F32, tag="std")
        nc.scalar.activation(std[:], var[:],
                             mybir.ActivationFunctionType.Sqrt, bias=eps[:])
        inv = work.tile([BE, 1], F32, tag="inv")
        nc.vector.reciprocal(inv[:], std[:])
        nbias = work.tile([BE, 1], F32, tag="nbias")
        nc.vector.tensor_mul(nbias[:], mean[:], inv[:])
        nc.scalar.mul(nbias[:], nbias[:], -1.0)

        xn = work.tile([BE, SEQ_LEN], F32, tag="xn")
        nc.scalar.activation(xn[:], xt_t[:],
                             mybir.ActivationFunctionType.Identity,
                             bias=nbias[:], scale=inv[:])

        # ---------------- phase B: transpose xn -> ft[j] = (t, be) --------
        ft = [work.tile([128, BE], MM_DT, tag=f"ft{j}", name=f"ft{j}")
              for j in range(L1CH)]
        for j in range(L1CH):
            pst = ps_tr.tile([128, 128], F32, tag="tr")
            nc.tensor.transpose(pst[:], xn[:, j * 128:(j + 1) * 128],
                                ident[:])
            nc.vector.tensor_copy(ft[j][:], pst[:])

        # ---------------- phase C: stages 1+2 per scale -------------------
        xdc = work.tile([BE, 2 * N_ORD], F32, tag="xdc")
        for s in (0, 1):
            L = (s + 1) * PRED_LEN
            lch = L // 128
            j0 = L1CH - lch                  # ft chunk offset for this scale
            # stage 1: Exf[be, nk] += ft[d].T @ W2[d]
            w2t = {}
            for part in ("re", "im"):
                w2t[part] = w2pool.tile([128, lch, NK], MM_DT,
                                        tag=f"w2{part}", name=f"w2t{part}")
                nc.sync.dma_start(
                    w2t[part][:],
                    w2[s, part].rearrange("(c p) f -> p c f", p=128))
            exf_sb = {}
            for part in ("re", "im"):
                exf_sb[part] = work.tile([BE, NK], MM_DT,
                                           tag=f"exfsb{part}",
                                           name=f"exfsb{part}")
            for part in ("re", "im"):
                for h in (0, 1):
                    eps_t = ps_exf.tile([BE, 512], F32, tag="exf",
                                        name="exfps")
                    for d in range(lch):
                        nc.tensor.matmul(
                            eps_t[:],
                            lhsT=ft[j0 + d][:],
                            rhs=w2t[part][:, d, h * 512:(h + 1) * 512],
                            start=(d == 0), stop=(d == lch - 1))
                    nc.vector.tensor_copy(
                        exf_sb[part][:, h * 512:(h + 1) * 512], eps_t[:])
            # transpose Exf -> (nk, be) chunks
            exf_T = {}
            for part in ("re", "im"):
                for i in range(NK // 128):
                    pst = ps_tr.tile([128, 128], MM_DT, tag="tr")
                    nc.tensor.transpose(
                        pst[:], exf_sb[part][:, i * 128:(i + 1) * 128],
                        ident_mm[:])
                    t = work.tile([128, BE], MM_DT, tag=f"exfT{part}{i}",
                                  name=f"exfT{part}{i}")
                    nc.vector.tensor_copy(t[:], pst[:])
                    exf_T[part, i] = t
            # stage 2: xdc[be, o] += sum_i exf_T[i].T @ w[i]
            wt = {}
            for part in ("re", "im"):
                wt[part] = wpool.tile([128, NK // 128, N_ORD], MM_DT,
                                      tag=f"w{part}", name=f"wt{part}")
                nc.sync.dma_start(
                    wt[part][:],
                    wsp[s, part].rearrange("(c p) f -> p c f", p=128))
            xdc_ps = ps_small.tile([BE, N_ORD], F32, tag="xdc")
            nmm = NK // 128
            for i in range(nmm):
                for part in ("re", "im"):
                    nc.tensor.matmul(
                        xdc_ps[:],
                        lhsT=exf_T[part, i][:],
                        rhs=wt[part][:, i, :],
                        start=(i == 0 and part == "re"),
                        stop=(i == nmm - 1 and part == "im"))
            nc.vector.tensor_copy(xdc[:, s * N_ORD:(s + 1) * N_ORD],
                                  xdc_ps[:])

        # ---------------- phase D: AllReduce partial xdc ------------------
        bounce_in = dram.tile([BE, 2 * N_ORD], F32, tag="bin")
        bounce_out = dram.tile([BE, 2 * N_ORD], F32, tag="bout")
        nc.gpsimd.dma_start(bounce_in[:], xdc[:])
        nc.gpsimd.collective_compute(
            "AllReduce",
            mybir.AluOpType.add,
            replica_groups=[list(range(N_CORES))],
            ins=[bounce_in.opt()],
            outs=[bounce_out.opt()],
        )
        xdcr = work.tile([BE, 2 * N_ORD], F32, tag="xdcr")
        nc.gpsimd.dma_start(xdcr[:], bounce_out[:])

        # ---------------- phase E: dec + un-norm --------------------------
        # broadcast mlp scalars across partitions via 1-row matmul
        mlpw_sb = const.tile([1, 2], F32, tag="mlpw")
        nc.sync.dma_start(mlpw_sb[:], mlpw[:, :])
        mlpb_sb = const.tile([1, 1], F32, tag="mlpb")
        nc.sync.dma_start(mlpb_sb[:], mlpb[:, :])
        ones = const.tile([1, 128], F32, tag="ones")
        nc.vector.memset(ones[:], 1.0)
        ps_w = ps_small.tile([128, 2], F32, tag="psmall")
        nc.tensor.matmul(ps_w[:], lhsT=ones[:], rhs=mlpw_sb[:])
        ws_sb = work.tile([128, 2], F32, tag="ws")
        nc.vector.tensor_copy(ws_sb[:], ps_w[:])
        ps_b = ps_small.tile([128, 1], F32, tag="psmall")
        nc.tensor.matmul(ps_b[:], lhsT=ones[:], rhs=mlpb_sb[:])
        bs_sb = work.tile([128, 1], F32, tag="bs")
        nc.vector.tensor_copy(bs_sb[:], ps_b[:])

        emt_sb = []
        for s in (0, 1):
            t = const.tile([128, 2, PSL], F32, tag=f"emt{s}", name=f"emtsb{s}")
            nc.sync.dma_start(t[:], emt[s].rearrange("(c p) f -> p c f",
                                                     p=128))
            emt_sb.append(t)

        # transpose xdc chunks, scaling by mlp_weight[s] on the way out
        dec_ps = ps_small.tile([BE, PSL], F32, tag="dec")
        first = True
        for s in (0, 1):
            for j in (0, 1):
                pst = ps_tr.tile([128, 128], F32, tag="tr")
                nc.tensor.transpose(
                    pst[:], xdcr[:, s * N_ORD + j * 128:
                                 s * N_ORD + (j + 1) * 128], ident[:])
                xt_sc = work.tile([128, BE], F32, tag="xdcT")
                nc.scalar.activation(xt_sc[:], pst[:],
                                     mybir.ActivationFunctionType.Copy,
                                     scale=ws_sb[:, s:s + 1])
                nc.tensor.matmul(dec_ps[:], lhsT=xt_sc[:],
                                 rhs=emt_sb[s][:, j, :],
                                 start=first, stop=(s == 1 and j == 1))
                first = False

        # out = dec * std + (mlp_bias * std + mean)
        bmu = work.tile([BE, 1], F32, tag="bmu")
        nc.vector.tensor_mul(bmu[:], bs_sb[:], std[:])
        nc.vector.tensor_add(bmu[:], bmu[:], mean[:])
        out_sb = work.tile([BE, PSL], F32, tag="out")
        nc.scalar.activation(out_sb[:], dec_ps[:],
                             mybir.ActivationFunctionType.Identity,
                             bias=bmu[:], scale=std[:])
        nc.sync.dma_start(out_dec[:, :], out_sb[:])


_NC = None


def _get_nc():
    global _NC
    if _NC is None:
        _NC = _build_nc()
    return _NC


# ---------------------------------------------------------------- host side
def _in_maps(x_enc, spec_w_real, spec_w_imag, mlp_weight, mlp_bias):
    consts = _get_consts()
    xt = np.ascontiguousarray(
        np.transpose(x_enc, (0, 2, 1)).reshape(BE, SEQ_LEN)).astype(
            np.float32, copy=False)
    mw = np.asarray(mlp_weight, np.float32).reshape(1, 2)
    mb = np.asarray(mlp_bias, np.float32).reshape(1, 1)
    maps = []
    for c in range(N_CORES):
        n0 = c * NSL
        m = {"xt": xt, "mlpw": mw, "mlpb": mb}
        for s in (0, 1):
            w2re, w2imneg, emT = consts[s]
            L = (s + 1) * PRED_LEN
            m[f"w2re{s}"] = np.ascontiguousarray(
                w2re[:, n0:n0 + NSL, :].reshape(L, NK)).astype(MM_NP)
            m[f"w2im{s}"] = np.ascontiguousarray(
                w2imneg[:, n0:n0 + NSL, :].reshape(L, NK)).astype(MM_NP)
            m[f"wre{s}"] = np.ascontiguousarray(
                spec_w_real[s, n0:n0 + NSL].transpose(0, 2, 1).reshape(
                    NK, N_ORD)).astype(MM_NP)
            m[f"wim{s}"] = np.ascontiguousarray(
                spec_w_imag[s, n0:n0 + NSL].transpose(0, 2, 1).reshape(
                    NK, N_ORD)).astype(MM_NP)
            m[f"emt{s}"] = np.ascontiguousarray(
                emT[:, c * PSL:(c + 1) * PSL]).astype(np.float32)
        maps.append(m)
    return maps


def kernel(x_enc, spec_w_real, spec_w_imag, mlp_weight, mlp_bias,
           _trace=False, _trace_kwargs=None):
    x_enc = np.asarray(x_enc, np.float32)
    spec_w_real = np.asarray(spec_w_real, np.float32)
    spec_w_imag = np.asarray(spec_w_imag, np.float32)
    maps = _in_maps(x_enc, spec_w_real, spec_w_imag, mlp_weight, mlp_bias)
    nc = _get_nc()
    res = run_bass_kernel_spmd(nc, maps, list(range(N_CORES)),
                               trace=_trace, **(_trace_kwargs or {}))
    parts = [res.results[c]["out_dec"].reshape(B_SZ, E_IN, PSL)
             for c in range(N_CORES)]
    out = np.concatenate(parts, axis=2).transpose(0, 2, 1)
    if _trace:
        return np.ascontiguousarray(out), res
    return np.ascontiguousarray(out)
